# revision 3
# baseline (speedup 1.0000x reference)
"""Biased multi-head cross-attention on 8 TRN2 NeuronCores — v3.

Same math/sharding as v2 (core c owns batch c//2, heads (c%2)*8..+8; output
rows [h*64,(h+1)*64) of each batch depend only on head h; no collectives).

v3 perf changes (v2 was ~640us on-device, PE-bound at 80% + 195us HAM-cold):
  - staging loads go through SWDGE cast-DMA straight to bf16; transposes use
    the regular-matmul trick (stationary=data chunk, moving=identity) instead
    of is_transpose f32, so they run ~1cyc/row warm and count for HAM.
  - exp is one ACT instruction per [128, 2x512] score tile (2 PSUM banks),
    halving ACT instruction overhead.
  - the two heads of a pair issue score matmuls adjacently -> row groups
    (0,0)/(64,0) run concurrently in the PE array (K=64 each).
  - out projection contracts K=128: partitions 0-63 hold atF[d,t], partitions
    64-127 hold atF[d,t+1] (one shifted SBUF->SBUF DMA per (pair,head)), so a
    stationary slice [:, 2jj::16] pairs j=2jj with j=2jj+1 in one matmul.
  - normalization: eager PSUM evacuation frees the attn bank in ~0.7us,
    then ones-column denominator -> reciprocal -> DMA hop to partition 0 ->
    gpsimd broadcast -> one DVE multiply, all off the PE critical path.
  - out projection runs per-pair: shift DMAs at the next pair's first score
    group, matmuls a few groups later; next pair's projections emit before
    the final attn drains so the PE never idles past the HAM window.
  - telescoped prologue: per-chunk q/k/v projection pieces emit as soon as
    their hs/kv row-tiles land; kv/hs load via HWDGE f32 + DVE cast in
    parallel with the SWDGE cast stream (weights, bias, Wo).
"""

import sys

for _p in ("/opt/trn_rl_repo", "/root/.axon_site/_ro/trn_rl_repo"):
    if _p not in sys.path:
        sys.path.insert(0, _p)

import numpy as np

B, T, S, E = 4, 1024, 2048, 1024
H, HD = 16, 64
NCORES = 8
NH = 8          # heads per core
NP = NH // 2    # head-pairs per core
P = 128
TB = T // 512   # t-blocks of 512
NSI = S // P    # s-tiles of 128
scaling = HD ** -0.5

_cache = {}


def _build_nc():
    import concourse.mybir as mybir
    import concourse.tile as tile
    from concourse import bacc
    from concourse.masks import make_identity

    f32 = mybir.dt.float32
    bf16 = mybir.dt.bfloat16
    Exp = mybir.ActivationFunctionType.Exp
    Ident = mybir.ActivationFunctionType.Identity

    nc = bacc.Bacc(None, target_bir_lowering=False)

    hs_p = nc.declare_dram_parameter("hs", [T, E], f32, isOutput=False)
    kv_p = nc.declare_dram_parameter("kv", [S, E], f32, isOutput=False)
    bias_p = nc.declare_dram_parameter("bias", [NH, T, S], f32, isOutput=False)
    Wq_p = nc.declare_dram_parameter("Wq", [NH * HD, E], f32, isOutput=False)
    Wk_p = nc.declare_dram_parameter("Wk", [NH * HD, E], f32, isOutput=False)
    Wv_p = nc.declare_dram_parameter("Wv", [NH * HD, E], f32, isOutput=False)
    Wo_p = nc.declare_dram_parameter("Wo", [E, E], f32, isOutput=False)
    bq_p = nc.declare_dram_parameter("bq", [NH * HD], f32, isOutput=False)
    bk_p = nc.declare_dram_parameter("bk", [NH * HD], f32, isOutput=False)
    bv_p = nc.declare_dram_parameter("bv", [NH * HD], f32, isOutput=False)
    bo_p = nc.declare_dram_parameter("bo", [E], f32, isOutput=False)
    out_p = nc.declare_dram_parameter("out", [NH * HD, E], f32, isOutput=True)

    halves = (slice(0, HD), slice(HD, P))

    with tile.TileContext(nc) as tc:
        with tc.tile_pool(name="statics", bufs=1) as statics:
            id_bf = statics.tile([P, P], bf16)
            make_identity(nc, id_bf[:])

            bq_s = statics.tile([P, NP], f32)   # bq_s[p,hp]=0.125*bq[hp*128+p]
            bk_s = statics.tile([P, NP], f32)
            bv_rep = statics.tile([P, NP, P], bf16)
            bo_rep = statics.tile([P, E], bf16)

            hsT = statics.tile([P, 8, T], bf16)        # hsT[e%128, e//128, t]
            kvT = statics.tile([P, 8, S], bf16)
            WqTa = statics.tile([P, NP, 8, P], bf16)   # [e%128, hp, e//128, m]
            WkTa = statics.tile([P, NP, 8, P], bf16)
            WvTa = statics.tile([P, NP, 8, P], bf16)
            WoT2 = statics.tile([P, 8, E], bf16)       # WoT2[c%128, c//128, e]

            # ---- small vectors ----
            with tc.tile_pool(name="vecs", bufs=1) as vecs:
                nc.sync.dma_start(bq_s[:], bq_p.rearrange("(hp p) -> p hp", p=P))
                nc.vector.tensor_scalar_mul(bq_s[:], bq_s[:], scaling)
                nc.sync.dma_start(bk_s[:], bk_p.rearrange("(hp p) -> p hp", p=P))
                bv_row = vecs.tile([1, NH * HD], bf16)
                nc.gpsimd.dma_start(bv_row[:], bv_p[None, :])
                for hp in range(NP):
                    nc.gpsimd.partition_broadcast(
                        bv_rep[:, hp, :], bv_row[0:1, hp * P:(hp + 1) * P])
                bo_row = vecs.tile([1, E], bf16)
                nc.gpsimd.dma_start(bo_row[:], bo_p[None, :])
                nc.gpsimd.partition_broadcast(bo_rep[:], bo_row[0:1, :])

            with tc.tile_pool(name="bias", bufs=2) as bias_pool:

                def bias_dma(hp, tb, g, half):
                    hl = hp * 2 + g
                    bt = bias_pool.tile(
                        [P, 2, S], bf16, tag=f"bias{g}{half}",
                        name=f"bt_{hl}_{tb}_{half}")
                    r = tb * 512 + half * 256
                    nc.gpsimd.dma_start(
                        bt[:], bias_p[hl, r:r + 256, :]
                        .rearrange("(a p) s -> p a s", p=P))
                    return bt

                # ------------ phase A: staging (bf16 transposes) ----------
                with tc.tile_pool(name="pairqkv", bufs=2) as pairqkv:
                  with tc.tile_pool(name="stg", bufs=3) as stg, \
                       tc.tile_pool(name="tp", bufs=4, space="PSUM") as tp_pool:

                    def transpose2(stb, dst_fn, nr=2):
                        # stb: SBUF [128, nr, E] bf16; PE transpose-trick per
                        # 128-chunk into dst_fn(r, jb) ([128, 4, 128] bf16).
                        for r in range(nr):
                            for jb in range(2):
                                tp = tp_pool.tile([P, 4, P], f32, tag="tp")
                                for a in range(4):
                                    c = jb * 4 + a
                                    nc.tensor.matmul(
                                        tp[:, a, :],
                                        stb[:, r, c * P:(c + 1) * P],
                                        id_bf[:], start=(a == 0), stop=(a == 3))
                                # ACT is idle during staging; keeping all
                                # evacuations there leaves DVE free for casts
                                # (no FIFO head-of-line between the two).
                                nc.scalar.copy(dst_fn(r, jb), tp[:])

                    def stage_sw(src_ap, dst_fn):
                        # SWDGE cast-DMA straight to bf16 (shares the queue
                        # with the bias stream).
                        st = stg.tile([P, 2, E], bf16, tag="st")
                        nc.gpsimd.dma_start(
                            st[:], src_ap.rearrange("(r p) e -> p r e", p=P))
                        transpose2(st, dst_fn)

                    def stage_hw1(src_ap, dst_fn):
                        # HWDGE f32 single-row-tile load + DVE cast.
                        sf = stg.tile([P, E], f32, tag="sf")
                        nc.sync.dma_start(sf[:], src_ap)
                        stb = stg.tile([P, E], bf16, tag="stb")
                        nc.vector.tensor_copy(stb[:], sf[:])
                        transpose2(stb[:, None, :], dst_fn, nr=1)

                    projected = {}

                    def proj_k_sb(hp, kTp, sb, psum):
                        ps = psum.tile([P, 512], f32, tag="pj")
                        for j in range(8):
                            nc.tensor.matmul(
                                ps[:], WkTa[:, hp, j, :],
                                kvT[:, j, sb * 512:(sb + 1) * 512],
                                start=(j == 0), stop=(j == 7))
                        nc.scalar.activation(
                            kTp[:, sb * 512:(sb + 1) * 512], ps[:], Ident,
                            bias=bk_s[:, hp:hp + 1])

                    def proj_v_si(hp, v_aug, si, psum):
                        ps = psum.tile([P, P], f32, tag="pj")
                        for j in range(8):
                            nc.tensor.matmul(
                                ps[:], kvT[:, j, si * P:(si + 1) * P],
                                WvTa[:, hp, j, :],
                                start=(j == 0), stop=(j == 7))
                        nc.vector.tensor_tensor(
                            v_aug[:, :, si, 0:HD],
                            ps.rearrange("p (g d) -> p g d", g=2),
                            bv_rep[:, hp, :].rearrange("p (g d) -> p g d", g=2),
                            mybir.AluOpType.add)

                    def proj_q_tb(hp, qTp, tb, psum):
                        ps = psum.tile([P, 512], f32, tag="pj")
                        for j in range(8):
                            nc.tensor.matmul(
                                ps[:], WqTa[:, hp, j, :],
                                hsT[:, j, tb * 512:(tb + 1) * 512],
                                start=(j == 0), stop=(j == 7))
                        nc.scalar.activation(
                            qTp[:, tb * 512:(tb + 1) * 512], ps[:], Ident,
                            bias=bq_s[:, hp:hp + 1], scale=scaling)

                    def proj_kv(hp, kTp, v_aug, psum):
                        nc.any.memset(v_aug[:, :, :, HD:HD + 1], 1.0)
                        for sb in range(S // 512):
                            proj_k_sb(hp, kTp, sb, psum)
                        for si in range(NSI):
                            proj_v_si(hp, v_aug, si, psum)

                    def proj_q(hp, qTp, psum):
                        for tb in range(TB):
                            proj_q_tb(hp, qTp, tb, psum)

                    # SWDGE stream order: Wq, Wk, Wv, bias00, Wo.
                    # HWDGE stream order: hs, then kv.
                    # PE order: W transposes, hs transposes + q-proj, kv
                    # transposes + k/v-proj, Wo transposes, then scores --
                    # every piece emitted right behind its data.
                    for Wp, dst in ((Wq_p, WqTa), (Wk_p, WkTa), (Wv_p, WvTa)):
                        for i2 in range(2):
                            stage_sw(Wp[i2 * 256:(i2 + 1) * 256, :],
                                     lambda r, jb, dst=dst, i2=i2: dst[
                                         :, i2 * 2 + r, jb * 4:(jb + 1) * 4, :])
                    pre = {(g, half): bias_dma(0, 0, g, half)
                           for g in range(2) for half in range(2)}
                    qTp0 = pairqkv.tile([P, T], bf16, tag="qTp")
                    kTp0 = pairqkv.tile([P, S], bf16, tag="kTp")
                    vaug0 = pairqkv.tile([P, 2, NSI, HD + 1], bf16,
                                         tag="vaug")
                    projected[0] = (qTp0, kTp0, vaug0)
                    nc.any.memset(vaug0[:, :, :, HD:HD + 1], 1.0)
                    for i in range(T // P):
                        stage_hw1(hs_p[i * P:(i + 1) * P, :],
                                  lambda r, jb, i=i: hsT[
                                      :, jb * 4:(jb + 1) * 4,
                                      i * P:(i + 1) * P])
                        if i % 4 == 3:
                            proj_q_tb(0, qTp0, i // 4, tp_pool)
                    for i in range(S // P):
                        stage_hw1(kv_p[i * P:(i + 1) * P, :],
                                  lambda r, jb, i=i: kvT[
                                      :, jb * 4:(jb + 1) * 4,
                                      i * P:(i + 1) * P])
                        proj_v_si(0, vaug0, i, tp_pool)
                        if i % 4 == 3:
                            proj_k_sb(0, kTp0, i // 4, tp_pool)
                    for i2 in range(E // 256):
                        stage_sw(Wo_p[i2 * 256:(i2 + 1) * 256, :],
                                 lambda r, jb, i2=i2: WoT2[
                                     :, jb * 4:(jb + 1) * 4,
                                     (i2 * 2 + r) * P:(i2 * 2 + r + 1) * P])

                  # ------------ phase B: per-pair main loop ---------------
                  with tc.tile_pool(name="atf", bufs=2) as atf_pool, \
                       tc.tile_pool(name="pt", bufs=4) as pt_pool, \
                       tc.tile_pool(name="nrm", bufs=2) as nrm_pool, \
                       tc.tile_pool(name="ob", bufs=1) as ob_pool, \
                       tc.tile_pool(name="proj", bufs=2, space="PSUM") as proj_pool, \
                       tc.tile_pool(name="sc", bufs=2, space="PSUM") as sc_pool, \
                       tc.tile_pool(name="at", bufs=2, space="PSUM") as at_pool:

                    bias_pre = {(0, 0): pre}

                    def prefetch_bias(hp, tb):
                        nxt = (hp, tb + 1) if tb + 1 < TB else (hp + 1, 0)
                        if nxt[0] < NP and nxt not in bias_pre:
                            bias_pre[nxt] = {
                                (g, half): bias_dma(nxt[0], nxt[1], g, half)
                                for g in range(2) for half in range(2)}

                    def proj_emit(hp):
                        qTp = pairqkv.tile([P, T], bf16, tag="qTp")
                        kTp = pairqkv.tile([P, S], bf16, tag="kTp")
                        v_aug = pairqkv.tile([P, 2, NSI, HD + 1], bf16,
                                             tag="vaug")
                        projected[hp] = (qTp, kTp, v_aug)
                        proj_kv(hp, kTp, v_aug, proj_pool)
                        proj_q(hp, qTp, proj_pool)

                    def shift_emit(atF, tb):
                        # upper copy for one t-block: u = t-1; odd u unread
                        u0 = tb * 512
                        for g in range(2):
                            nc.sync.dma_start(
                                atF[HD:P, g, u0:u0 + 511],
                                atF[0:HD, g, u0 + 1:u0 + 512])

                    def outproj_mms(hp, atF):
                        for n in range(2):
                            po = proj_pool.tile([P, 512], f32, tag="pj")
                            for jj in range(8):
                                nc.tensor.matmul(
                                    po[:],
                                    atF[:, :, 2 * jj::16],
                                    WoT2[:, jj, n * 512:(n + 1) * 512],
                                    start=(jj == 0), stop=(jj == 7))
                            ob = ob_pool.tile([P, 512], f32, tag="ob")
                            nc.vector.tensor_tensor(
                                ob[:], po[:],
                                bo_rep[:, n * 512:(n + 1) * 512],
                                mybir.AluOpType.add)
                            nc.sync.dma_start(
                                out_p[hp * P:(hp + 1) * P,
                                      n * 512:(n + 1) * 512],
                                ob[:])

                    prev = None  # (hp, atF) awaiting out-projection
                    for hp in range(NP):
                        qTp, kTp, v_aug = projected.pop(hp)
                        atF = atf_pool.tile([P, 2, T], bf16, tag="atf")
                        pend = []
                        last = hp == NP - 1

                        def norm_g(at_ps_g, tsl_l, g):
                            # eager full-tile copy frees the PSUM bank fast;
                            # the slow reciprocal chain runs from SBUF off
                            # the PE critical path.
                            ataw = nrm_pool.tile([HD + 1, 512], f32,
                                                 tag="ataw")
                            nc.vector.tensor_copy(ataw[:], at_ps_g[:])
                            recrow = nrm_pool.tile([HD + 1, 512], f32,
                                                   tag="recrow")
                            nc.vector.reciprocal(
                                recrow[HD:HD + 1, :], ataw[HD:HD + 1, :])
                            nc.sync.dma_start(recrow[0:1, :],
                                              recrow[HD:HD + 1, :])
                            recb = nrm_pool.tile([HD, 512], f32, tag="recb")
                            nc.gpsimd.partition_broadcast(
                                recb[:], recrow[0:1, :])
                            nc.vector.tensor_tensor(
                                atF[0:HD, g, tsl_l], ataw[0:HD, :],
                                recb[:], mybir.AluOpType.mult)

                        def drain_one():
                            at_ps_l, tsl_l, g, sp2, pt2 = pend.pop(0)
                            for h2 in range(2):
                                si2 = 2 * sp2 + h2
                                nc.tensor.matmul(
                                    at_ps_l[g][:], v_aug[:, g, si2, :],
                                    pt2[:, h2, :],
                                    start=(si2 == 0), stop=(si2 == NSI - 1))
                            if sp2 == NSI // 2 - 1:
                                norm_g(at_ps_l[g], tsl_l, g)

                        # ---- scores + softmax + attn ----
                        for tb in range(TB):
                            tsl = slice(tb * 512, (tb + 1) * 512)
                            bts = bias_pre.pop((hp, tb))
                            prefetch_bias(hp, tb)
                            at_ps = [at_pool.tile([HD + 1, 512], f32, tag="at",
                                                  name=f"at_{hp}_{g}_{tb}")
                                     for g in range(2)]
                            for sp in range(NSI // 2):
                                scs = []
                                for g in range(2):
                                    sc = sc_pool.tile([P, 2, 512], f32,
                                                      tag="sc")
                                    for h in range(2):
                                        si = 2 * sp + h
                                        for a in range(4):
                                            src = bts[(g, a // 2)]
                                            nc.tensor.matmul(
                                                sc[:, h, a * P:(a + 1) * P],
                                                src[:, a % 2,
                                                    si * P:(si + 1) * P],
                                                id_bf[:],
                                                start=(a == 0), stop=False)
                                    scs.append(sc)
                                # row-tiled score matmuls: g0 rows 0-63,
                                # g1 rows 64-127 -> concurrent in PE
                                for h in range(2):
                                    si = 2 * sp + h
                                    for g in range(2):
                                        nc.tensor.matmul(
                                            scs[g][:, h, :],
                                            kTp[halves[g], si * P:(si + 1) * P],
                                            qTp[halves[g], tsl],
                                            start=False, stop=True)
                                for g in range(2):
                                    pt = pt_pool.tile(
                                        [P, 2, 512], bf16, tag="pt",
                                        name=f"pt_{hp}_{tb}_{sp}_{g}")
                                    nc.scalar.activation(pt[:], scs[g][:], Exp)
                                    pend.append((at_ps, tsl, g, sp, pt))
                                # previous pair's out-projection: shift DMAs
                                # at sp0, matmuls a few groups later (after
                                # its tb1 norm chain has surely finished).
                                if prev is not None and tb == 0:
                                    if sp == 0:
                                        shift_emit(prev[1], 0)
                                        shift_emit(prev[1], 1)
                                    elif sp == 5:
                                        outproj_mms(*prev)
                                        prev = None
                                while len(pend) > 2:
                                    drain_one()
                            # next pair's projections fill the PE pipeline
                            # while this tb's last exps finish (keeps HAM
                            # warm across the pair boundary).
                            if tb == TB - 1 and hp + 1 < NP:
                                proj_emit(hp + 1)
                        while pend:
                            drain_one()
                        prev = (hp, atF)

                    shift_emit(prev[1], 0)
                    shift_emit(prev[1], 1)
                    outproj_mms(*prev)

    nc.compile()
    return nc


def get_nc():
    if "nc" not in _cache:
        _cache["nc"] = _build_nc()
    return _cache["nc"]


def make_in_maps(inputs):
    f = lambda x: np.asarray(x, dtype=np.float32)
    hs = f(inputs["hidden_states"])
    kv = f(inputs["key_value_states"])
    bias = f(inputs["bias"])
    Wq, bq = f(inputs["Wq"]), f(inputs["bq"])
    Wk, bk = f(inputs["Wk"]), f(inputs["bk"])
    Wv, bv = f(inputs["Wv"]), f(inputs["bv"])
    Wo, bo = f(inputs["Wo"]), f(inputs["bo"])
    in_maps = []
    for c in range(NCORES):
        b, h0 = c // 2, (c % 2) * NH
        r = slice(h0 * HD, (h0 + NH) * HD)
        in_maps.append({
            "hs": hs[b], "kv": kv[b], "bias": bias[b, h0:h0 + NH],
            "Wq": Wq[r], "Wk": Wk[r], "Wv": Wv[r], "Wo": Wo,
            "bq": bq[r], "bk": bk[r], "bv": bv[r], "bo": bo,
        })
    return in_maps


def assemble(results):
    out = np.empty((B, T, E), dtype=np.float32)
    for c in range(NCORES):
        b, h0 = c // 2, (c % 2) * NH
        out[b, h0 * HD:(h0 + NH) * HD, :] = results[c]["out"]
    return out


def kernel(**inputs):
    from concourse.bass_utils import run_bass_kernel_spmd

    nc = get_nc()
    res = run_bass_kernel_spmd(nc, make_in_maps(inputs), core_ids=list(range(NCORES)))
    return assemble(res.results)
